# revision 13
# baseline (speedup 1.0000x reference)
"""ConvexUpsampler Trainium2 kernel.

Computes, per batch element b (one NeuronCore each, 8 cores):
  x    = relu(conv3x3(feat, w1) + b1)            # [256, 96, 96]
  m    = conv1x1(x, w2) + b2                     # [576, 96, 96]
  mask = softmax over k of m.reshape(9, 64, ...) # k = 3x3 tap index
  up   = sum_k mask[k,pq,hw] * unfold(flow)[c,k,hw] * 8
  out  = pixel-shuffle(up)                       # [2, 768, 768]

Strategy: data-parallel over batch (8 cores).  Convs run on the PE as
accumulated matmuls (fp32r operands).  conv2 is "swapped" so its PSUM
output has pixels on partitions, which lets the softmax-weighted convex
combination run as wide DVE ops with the unfolded flow entering via
free-dim broadcast APs.  Flow unfolding + all weight re-layouts are done
host-side in numpy (cheap, tiny tensors).
"""

import os
import sys
from contextlib import ExitStack

sys.path.insert(0, "/opt/trn_rl_repo")

import numpy as np

import concourse.bass as bass  # noqa: E402
import concourse.tile as tile  # noqa: E402
from concourse import bacc, mybir  # noqa: E402
from concourse.bass_utils import run_bass_kernel_spmd  # noqa: E402

F32 = mybir.dt.float32
F32R = mybir.dt.float32r

B = 8
C = 256
H = W = 96
UP = 8
PIX = H * W          # 9216
GW = 98              # padded grid width
NBAND = 24           # bands of 4 output rows
BAND_N = 4 * GW      # matmul free size for conv1 (392)
NCHUNK = PIX // 128  # 72 pixel chunks of 128

# matmul operand dtype knob: "f32r" (full speed), "bf16" (full speed, lower
# precision), or "f32" (4x slower, exact).  The whole producer chain (DRAM
# decl -> DMA -> SBUF tile -> ACT output) is declared in this dtype: the
# walrus verifier requires fp32r matmul operands to be *produced* as fp32r.
BF16 = mybir.dt.bfloat16
MM_NAME = os.environ.get("CONVUP_MM_DT", "f32r")
MM_DT = {"f32r": F32R, "f32": F32, "bf16": BF16}[MM_NAME]


def _mm_np(a):
    """Host-side array in the dtype matching the MM_DT DRAM declarations."""
    if MM_NAME == "bf16":
        import ml_dtypes

        return np.ascontiguousarray(a).astype(ml_dtypes.bfloat16)
    return np.ascontiguousarray(a, np.float32)


def _chunk_segments(j):
    """Split pixel chunk j (pixels 128j..128j+127, h-major) into runs with a
    single output row each: (i0, n, h, w0)."""
    segs = []
    i = 0
    while i < 128:
        pix = 128 * j + i
        h, w0 = divmod(pix, W)
        n = min(128 - i, W - w0)
        segs.append((i, n, h, w0))
        i += n
    return segs


def _build(with_b2: bool):
    nc = bacc.Bacc("TRN2", target_bir_lowering=False, debug=False)
    featp = nc.dram_tensor("featp", [128, 2, 100 * GW], MM_DT, kind="ExternalInput").ap()
    w1s = nc.dram_tensor("w1s", [128, 2, 9, 2, 128], MM_DT, kind="ExternalInput").ap()
    w2s = nc.dram_tensor("w2s", [128, 2, 576], MM_DT, kind="ExternalInput").ap()
    b1s = nc.dram_tensor("b1s", [128, 2], F32, kind="ExternalInput").ap()
    fdat = nc.dram_tensor("fdat", [128, NCHUNK, 18], F32, kind="ExternalInput").ap()
    b2s = None
    if with_b2:
        b2s = nc.dram_tensor("b2s", [1, 576], MM_DT, kind="ExternalInput").ap()
    out = nc.dram_tensor("out", [2, 768, 768], F32, kind="ExternalOutput").ap()
    # out viewed as [c, hh, p, ww, q] for the pixel-shuffle scatter store
    out_v = out.rearrange("c (hh p) (ww q) -> c hh p ww q", p=UP, q=UP)

    with tile.TileContext(nc) as tc, ExitStack() as ctx:
        cpool = ctx.enter_context(tc.tile_pool(name="const", bufs=1))
        xpool = ctx.enter_context(tc.tile_pool(name="x", bufs=3))
        epool = ctx.enter_context(tc.tile_pool(name="e", bufs=3))
        ppool = ctx.enter_context(tc.tile_pool(name="prod", bufs=2))
        npool = ctx.enter_context(tc.tile_pool(name="num", bufs=2))
        dpool = ctx.enter_context(tc.tile_pool(name="d", bufs=2))
        upool = ctx.enter_context(tc.tile_pool(name="up", bufs=3))
        psum1 = ctx.enter_context(tc.tile_pool(name="ps1", bufs=4, space="PSUM"))
        psum2 = ctx.enter_context(tc.tile_pool(name="ps2", bufs=4, space="PSUM"))

        feat_sb = cpool.tile([128, 2, 100 * GW], MM_DT, tag="feat")
        # split the big feat load so early conv1 bands start sooner
        for kc in range(2):
            for s0 in range(0, 100 * GW, 25 * GW):
                nc.sync.dma_start(
                    feat_sb[:, kc, s0 : s0 + 25 * GW],
                    featp[:, kc, s0 : s0 + 25 * GW],
                )
        w1_sb = cpool.tile([128, 2, 9, 2, 128], MM_DT, tag="w1")
        nc.sync.dma_start(w1_sb[:], w1s[:])
        w2_sb = cpool.tile([128, 2, 576], MM_DT, tag="w2")
        nc.sync.dma_start(w2_sb[:], w2s[:])
        b1_sb = cpool.tile([128, 2], F32, tag="b1")
        nc.sync.dma_start(b1_sb[:], b1s[:])
        f_sb = cpool.tile([128, NCHUNK, 18], F32, tag="fdat")
        nc.sync.dma_start(f_sb[:], fdat[:])
        if with_b2:
            b2_sb = cpool.tile([1, 576], MM_DT, tag="b2")
            nc.sync.dma_start(b2_sb[:], b2s[:])
            ones_sb = cpool.tile([1, 128], MM_DT, tag="ones")
            nc.vector.memset(ones_sb[:], 1.0)

        def conv1_band(hb):
            r0 = 4 * hb + 1  # first output grid row of this band
            # --- conv1: 3x3x256->256 over 4 rows (padded width) ---
            xb = []
            for mc in range(2):
                ps = psum1.tile([128, BAND_N], F32, tag="ps1")
                first = True
                for kc in range(2):
                    for tap in range(9):
                        dh, dw = divmod(tap, 3)
                        s = (r0 + dh) * GW + dw - 1
                        nc.tensor.matmul(
                            ps[:],
                            lhsT=w1_sb[:, kc, tap, mc, :],
                            rhs=feat_sb[:, kc, s : s + BAND_N],
                            start=first,
                            stop=(kc == 1 and tap == 8),
                        )
                        first = False
                # relu(x + b1), compacting 98-wide padded rows to 96
                xt = xpool.tile([128, 4 * W], MM_DT, tag=f"x{mc}")
                nc.scalar.activation(
                    out=xt[:].rearrange("p (r c) -> p r c", c=W),
                    in_=ps[:].rearrange("p (r c) -> p r c", c=GW)[:, :, 1 : W + 1],
                    func=mybir.ActivationFunctionType.Relu,
                    bias=b1_sb[:, mc : mc + 1],
                )
                xb.append(xt)
            return xb

        def tail_band(hb, xb):
            # --- conv2 (swapped: pixels on partitions) + softmax + convex ---
            e_b = epool.tile([128, 3, 576], F32, tag="e")
            for t in range(3):
                for half in range(2):
                    ps2 = psum2.tile([128, 288], F32, tag="ps2")
                    for kc in range(2):
                        nc.tensor.matmul(
                            ps2[:],
                            lhsT=xb[kc][:, t * 128 : (t + 1) * 128],
                            rhs=w2_sb[:, kc, half * 288 : (half + 1) * 288],
                            start=(kc == 0),
                            stop=(kc == 1 and not with_b2),
                        )
                    if with_b2:
                        nc.tensor.matmul(
                            ps2[:],
                            lhsT=ones_sb[:, :],
                            rhs=b2_sb[:, half * 288 : (half + 1) * 288],
                            start=False,
                            stop=True,
                        )
                    nc.scalar.activation(
                        out=e_b[:, t, half * 288 : (half + 1) * 288],
                        in_=ps2[:],
                        func=mybir.ActivationFunctionType.Exp,
                    )
            # band-wide views [128, 3, 64, 9]
            e4 = e_b[:].rearrange("p t (q k) -> p t q k", k=9)

            num_b = npool.tile([128, 3, 2, 64], F32, tag="num")
            for c in range(2):
                pr = ppool.tile([128, 3, 576], F32, tag=f"prod{c}")
                prv = pr[:].rearrange("p t (q k) -> p t q k", k=9)
                fb = f_sb[:, 3 * hb : 3 * hb + 3, None, c * 9 : c * 9 + 9]
                fb = fb.to_broadcast((128, 3, 64, 9))
                # products on GPSIMD (frees the vector engine)
                mul_eng = (
                    nc.vector if os.environ.get("CONVUP_NO_GPS") else nc.gpsimd
                )
                mul_eng.tensor_tensor(
                    out=prv, in0=e4, in1=fb, op=mybir.AluOpType.mult
                )
                nc.vector.tensor_reduce(
                    out=num_b[:, :, c, :],
                    in_=prv,
                    axis=mybir.AxisListType.X,
                    op=mybir.AluOpType.add,
                )
            ds = dpool.tile([128, 3, 64], F32, tag="dsum")
            nc.vector.tensor_reduce(
                out=ds[:], in_=e4, axis=mybir.AxisListType.X,
                op=mybir.AluOpType.add,
            )
            di = dpool.tile([128, 3, 64], F32, tag="dinv")
            nc.vector.reciprocal_approx_fast(di[:], ds[:])

            up_b = upool.tile([128, 3, 2, 64], F32, tag="up")
            nc.vector.tensor_tensor(
                out=up_b[:],
                in0=num_b[:],
                in1=di[:, :, None, :].to_broadcast((128, 3, 2, 64)),
                op=mybir.AluOpType.mult,
            )

            for t in range(3):
                j = 3 * hb + t
                for i0, n, h, w0 in _chunk_segments(j):
                    for c in range(2):
                        nc.sync.dma_start(
                            out_v[c, h, :, w0 : w0 + n, :].rearrange(
                                "p w q -> w p q"
                            ),
                            up_b[i0 : i0 + n, t, c, :].rearrange(
                                "w (p q) -> w p q", q=UP
                            ),
                        )

        # Software pipeline: emit band hb's conv1 before band hb-1's tail so
        # the (in-order) PE stream never stalls waiting on the ACT/DVE tail.
        xb_prev = None
        for hb in range(NBAND):
            xb = conv1_band(hb)
            if xb_prev is not None:
                tail_band(hb - 1, xb_prev)
            xb_prev = xb
        tail_band(NBAND - 1, xb_prev)
    nc.compile()
    return nc


_NC_CACHE = {}


def _get_nc(with_b2: bool):
    if with_b2 not in _NC_CACHE:
        _NC_CACHE[with_b2] = _build(with_b2)
    return _NC_CACHE[with_b2]


def _prep_core(feat_b, flow_b):
    """Per-core input arrays from one batch element."""
    # feat: [256, 96, 96] -> padded grid [128p, 2kc, 100*98] with one extra
    # leading zero row (grid row r lives at flat (r+1)*98).
    featp = np.zeros((2, 128, 100 * GW), np.float32)
    grid = np.zeros((C, GW, GW), np.float32)
    grid[:, 1 : H + 1, 1 : W + 1] = feat_b
    featp[:, :, GW : GW + GW * GW] = grid.reshape(2, 128, GW * GW)
    featp = _mm_np(featp.transpose(1, 0, 2))

    # flow unfold (zero-padded 3x3 shifts), pre-scaled by UP=8:
    # fdat[i, j, c*9+k] = 8 * flow_pad[c, h+kh, w+kw] for pixel j*128+i
    fpad = np.zeros((2, H + 2, W + 2), np.float32)
    fpad[:, 1:-1, 1:-1] = flow_b
    shifts = np.stack(
        [fpad[:, i : i + H, j : j + W] for i in range(3) for j in range(3)],
        axis=1,
    )  # [2, 9, 96, 96]
    fdat = (8.0 * shifts).transpose(2, 3, 0, 1).reshape(PIX, 18)
    fdat = np.ascontiguousarray(fdat.reshape(NCHUNK, 128, 18).transpose(1, 0, 2))
    return featp, fdat


def _weight_args(inputs):
    """Re-laid-out weight arrays shared across cores."""
    w1 = np.asarray(inputs["w1"], np.float32)
    b1 = np.asarray(inputs["b1"], np.float32)
    w2 = np.asarray(inputs["w2"], np.float32)
    b2 = np.asarray(inputs["b2"], np.float32)
    # w1s[ci_in, kc, tap, mc, co_in] = w1[mc*128+co, kc*128+ci, kh, kw]
    t = w1.transpose(1, 2, 3, 0).reshape(2, 128, 3, 3, 2, 128)
    w1s = _mm_np(t.transpose(1, 0, 2, 3, 4, 5)).reshape(
        128, 2, 9, 2, 128
    )
    # w2 columns reordered to ch' = pq*9 + k (from ch = k*64 + pq)
    chp = np.arange(576)
    perm = (chp % 9) * 64 + chp // 9
    w2m = w2[:, :, 0, 0][perm]  # [576, 256]
    w2s = _mm_np(w2m.T.reshape(2, 128, 576).transpose(1, 0, 2))
    b1s = np.ascontiguousarray(b1.reshape(2, 128).T)
    args = {"w1s": w1s, "w2s": w2s, "b1s": b1s}
    if np.any(b2):
        args["b2s"] = _mm_np(b2[perm].reshape(1, 576))
    return args


def kernel(flow_lr, feat, w1, b1, w2, b2):
    flow_lr = np.asarray(flow_lr, np.float32)
    feat = np.asarray(feat, np.float32)
    inputs = {"w1": w1, "b1": b1, "w2": w2, "b2": b2}
    w_args = _weight_args(inputs)
    with_b2 = "b2s" in w_args

    nc = _get_nc(with_b2)
    in_maps = []
    for b in range(B):
        featp, fdat = _prep_core(feat[b], flow_lr[b])
        m = dict(w_args)
        m.update({"featp": featp, "fdat": fdat})
        in_maps.append(m)

    res = run_bass_kernel_spmd(nc, in_maps, list(range(B)))
    return np.stack([res.results[b]["out"] for b in range(B)]).astype(np.float32)


# revision 23
# speedup vs baseline: 404.5656x; 404.5656x over previous
"""ConvexUpsampler Trainium2 kernel.

Computes, per batch element b (one NeuronCore each, 8 cores):
  x    = relu(conv3x3(feat, w1) + b1)            # [256, 96, 96]
  m    = conv1x1(x, w2) + b2                     # [576, 96, 96]
  mask = softmax over k of m.reshape(9, 64, ...) # k = 3x3 tap index
  up   = sum_k mask[k,pq,hw] * unfold(flow)[c,k,hw] * 8
  out  = pixel-shuffle(up)                       # [2, 768, 768]

Strategy: data-parallel over batch (8 cores).  Convs run on the PE as
accumulated matmuls (fp32r operands).  conv2 is "swapped" so its PSUM
output has pixels on partitions, which lets the softmax-weighted convex
combination run as wide DVE ops with the unfolded flow entering via
free-dim broadcast APs.  Flow unfolding + all weight re-layouts are done
host-side in numpy (cheap, tiny tensors).
"""

import os
import sys
from contextlib import ExitStack

sys.path.insert(0, "/opt/trn_rl_repo")

import numpy as np

import concourse.bass as bass  # noqa: E402
import concourse.tile as tile  # noqa: E402
from concourse import bacc, mybir  # noqa: E402
from concourse.bass_utils import run_bass_kernel_spmd  # noqa: E402

F32 = mybir.dt.float32
F32R = mybir.dt.float32r

B = 8
C = 256
H = W = 96
UP = 8
PIX = H * W          # 9216
GW = 98              # padded grid width
NBAND = 24           # bands of 4 output rows
BAND_N = 4 * GW      # matmul free size for conv1 (392)
NCHUNK = PIX // 128  # 72 pixel chunks of 128

# matmul operand dtype knob: "f32r" (full speed), "bf16" (full speed, lower
# precision), or "f32" (4x slower, exact).  The whole producer chain (DRAM
# decl -> DMA -> SBUF tile -> ACT output) is declared in this dtype: the
# walrus verifier requires fp32r matmul operands to be *produced* as fp32r.
BF16 = mybir.dt.bfloat16
MM_NAME = os.environ.get("CONVUP_MM_DT", "f32r")
MM_DT = {"f32r": F32R, "f32": F32, "bf16": BF16}[MM_NAME]


def _mm_np(a):
    """Host-side array in the dtype matching the MM_DT DRAM declarations."""
    if MM_NAME == "bf16":
        import ml_dtypes

        return np.ascontiguousarray(a).astype(ml_dtypes.bfloat16)
    return np.ascontiguousarray(a, np.float32)


def _chunk_segments(j):
    """Split pixel chunk j (pixels 128j..128j+127, h-major) into runs with a
    single output row each: (i0, n, h, w0)."""
    segs = []
    i = 0
    while i < 128:
        pix = 128 * j + i
        h, w0 = divmod(pix, W)
        n = min(128 - i, W - w0)
        segs.append((i, n, h, w0))
        i += n
    return segs


def _build(with_b2: bool, reps: int = 1, with_b1: bool = True):
    nc = bacc.Bacc("TRN2", target_bir_lowering=False, debug=False)
    featp = nc.dram_tensor("featp", [128, 2, 100 * GW], MM_DT, kind="ExternalInput").ap()
    w1s = nc.dram_tensor("w1s", [128, 2, 9, 2, 128], MM_DT, kind="ExternalInput").ap()
    w2s = nc.dram_tensor("w2s", [128, 2, 576], MM_DT, kind="ExternalInput").ap()
    b1s = nc.dram_tensor("b1s", [128, 2], F32, kind="ExternalInput").ap()
    fdat = nc.dram_tensor("fdat", [128, NCHUNK, 18], F32, kind="ExternalInput").ap()
    b2s = None
    if with_b2:
        b2s = nc.dram_tensor("b2s", [1, 576], MM_DT, kind="ExternalInput").ap()
    out = nc.dram_tensor("out", [2, 768, 768], F32, kind="ExternalOutput").ap()
    # out viewed as [c, hh, p, ww, q] for the pixel-shuffle scatter store
    out_v = out.rearrange("c (hh p) (ww q) -> c hh p ww q", p=UP, q=UP)

    with tile.TileContext(nc) as tc, ExitStack() as ctx:
        cpool = ctx.enter_context(tc.tile_pool(name="const", bufs=1))
        xpool = ctx.enter_context(tc.tile_pool(name="x", bufs=3))
        epool = ctx.enter_context(tc.tile_pool(name="e", bufs=3))
        ppool = ctx.enter_context(tc.tile_pool(name="prod", bufs=2))
        npool = ctx.enter_context(tc.tile_pool(name="num", bufs=2))
        dpool = ctx.enter_context(tc.tile_pool(name="d", bufs=2))
        upool = ctx.enter_context(tc.tile_pool(name="up", bufs=3))
        psum1 = ctx.enter_context(tc.tile_pool(name="ps1", bufs=2, space="PSUM"))
        psum2 = ctx.enter_context(tc.tile_pool(name="ps2", bufs=2, space="PSUM"))

        feat_sb = cpool.tile([128, 2, 100 * GW], MM_DT, tag="feat")
        # split the big feat load so early conv1 bands start sooner
        for kc in range(2):
            for s0 in range(0, 100 * GW, 25 * GW):
                nc.sync.dma_start(
                    feat_sb[:, kc, s0 : s0 + 25 * GW],
                    featp[:, kc, s0 : s0 + 25 * GW],
                )
        w1_sb = cpool.tile([128, 2, 9, 2, 128], MM_DT, tag="w1")
        nc.sync.dma_start(w1_sb[:], w1s[:])
        w2_sb = cpool.tile([128, 2, 576], MM_DT, tag="w2")
        nc.sync.dma_start(w2_sb[:], w2s[:])
        b1_sb = cpool.tile([128, 2], F32, tag="b1")
        nc.sync.dma_start(b1_sb[:], b1s[:])
        f_sb = cpool.tile([128, NCHUNK, 18], F32, tag="fdat")
        nc.sync.dma_start(f_sb[:], fdat[:])
        if with_b2:
            b2_sb = cpool.tile([1, 576], MM_DT, tag="b2")
            nc.sync.dma_start(b2_sb[:], b2s[:])
            ones_sb = cpool.tile([1, 128], MM_DT, tag="ones")
            nc.vector.memset(ones_sb[:], 1.0)

        def conv1_band(hb):
            r0 = 4 * hb + 1  # first output grid row of this band
            # --- conv1: 3x3x256->256 over 4 rows (padded width) ---
            # both mc halves accumulate into one 2-bank psum tile
            ps = psum1.tile([128, 2, 512], F32, tag="ps1")
            for mc in range(2):
                first = True
                for kc in range(2):
                    for tap in range(9):
                        dh, dw = divmod(tap, 3)
                        s = (r0 + dh) * GW + dw - 1
                        nc.tensor.matmul(
                            ps[:, mc, :BAND_N],
                            lhsT=w1_sb[:, kc, tap, mc, :],
                            rhs=feat_sb[:, kc, s : s + BAND_N],
                            start=first,
                            stop=(kc == 1 and tap == 8),
                        )
                        first = False
            # relu(x + b1), compacting 98-wide padded rows to 96
            xt = xpool.tile([128, 2, 4 * W], MM_DT, tag="x")
            psv = ps[:, :, :BAND_N].rearrange("p m (r c) -> p m r c", c=GW)[
                :, :, :, 1 : W + 1
            ]
            if with_b1:
                for mc in range(2):
                    nc.scalar.activation(
                        out=xt[:, mc].rearrange("p (r c) -> p r c", c=W),
                        in_=psv[:, mc],
                        func=mybir.ActivationFunctionType.Relu,
                        bias=b1_sb[:, mc : mc + 1],
                    )
            else:
                nc.scalar.activation(
                    out=xt[:].rearrange("p m (r c) -> p m r c", c=W),
                    in_=psv,
                    func=mybir.ActivationFunctionType.Relu,
                )
            return xt

        def tail_band(hb, xb):
            # --- conv2 (swapped: pixels on partitions) + softmax + convex ---
            e_b = epool.tile([128, 3, 576], F32, tag="e")
            for t in range(3):
                ps2 = psum2.tile([128, 2, 512], F32, tag="ps2")
                for half in range(2):
                    for kc in range(2):
                        nc.tensor.matmul(
                            ps2[:, half, :288],
                            lhsT=xb[:, kc, t * 128 : (t + 1) * 128],
                            rhs=w2_sb[:, kc, half * 288 : (half + 1) * 288],
                            start=(kc == 0),
                            stop=(kc == 1 and not with_b2),
                        )
                    if with_b2:
                        nc.tensor.matmul(
                            ps2[:, half, :288],
                            lhsT=ones_sb[:, :],
                            rhs=b2_sb[:, half * 288 : (half + 1) * 288],
                            start=False,
                            stop=True,
                        )
                nc.scalar.activation(
                    out=e_b[:, t, :].rearrange("p (h n) -> p h n", h=2),
                    in_=ps2[:, :, :288],
                    func=mybir.ActivationFunctionType.Exp,
                )
            # band-wide views [128, 3, 64, 9]
            e4 = e_b[:].rearrange("p t (q k) -> p t q k", k=9)

            num_b = npool.tile([128, 3, 2, 64], F32, tag="num")
            for c in range(2):
                pr = ppool.tile([128, 3, 576], F32, tag=f"prod{c}")
                prv = pr[:].rearrange("p t (q k) -> p t q k", k=9)
                fb = f_sb[:, 3 * hb : 3 * hb + 3, None, c * 9 : c * 9 + 9]
                fb = fb.to_broadcast((128, 3, 64, 9))
                # products on GPSIMD (frees the vector engine)
                mul_eng = (
                    nc.vector if os.environ.get("CONVUP_NO_GPS") else nc.gpsimd
                )
                mul_eng.tensor_tensor(
                    out=prv, in0=e4, in1=fb, op=mybir.AluOpType.mult
                )
                nc.vector.tensor_reduce(
                    out=num_b[:, :, c, :],
                    in_=prv,
                    axis=mybir.AxisListType.X,
                    op=mybir.AluOpType.add,
                )
            ds = dpool.tile([128, 3, 64], F32, tag="dsum")
            nc.vector.tensor_reduce(
                out=ds[:], in_=e4, axis=mybir.AxisListType.X,
                op=mybir.AluOpType.add,
            )
            di = dpool.tile([128, 3, 64], F32, tag="dinv")
            nc.vector.reciprocal_approx_fast(di[:], ds[:])

            up_b = upool.tile([128, 3, 2, 64], F32, tag="up")
            nc.vector.tensor_tensor(
                out=up_b[:],
                in0=num_b[:],
                in1=di[:, :, None, :].to_broadcast((128, 3, 2, 64)),
                op=mybir.AluOpType.mult,
            )

            if os.environ.get("CONVUP_LINEAR_STORE"):
                # debug knob: contiguous (wrong-layout) store to measure the
                # cost of the pixel-shuffle scatter
                of = out.rearrange("c h w -> (c h w)")
                nc.sync.dma_start(
                    of[hb * 49152 : (hb + 1) * 49152].rearrange(
                        "(p f) -> p f", f=384
                    ),
                    up_b[:].rearrange("p t c q -> p (t c q)"),
                )
                return
            for t in range(3):
                j = 3 * hb + t
                for i0, n, h, w0 in _chunk_segments(j):
                    for c in range(2):
                        nc.sync.dma_start(
                            out_v[c, h, :, w0 : w0 + n, :].rearrange(
                                "p w q -> w p q"
                            ),
                            up_b[i0 : i0 + n, t, c, :].rearrange(
                                "w (p q) -> w p q", q=UP
                            ),
                        )

        # Software pipeline: emit band hb's conv1 before band hb-1's tail so
        # the (in-order) PE stream never stalls waiting on the ACT/DVE tail.
        # reps > 1 repeats the whole computation (timing tool only).
        for _ in range(reps):
            xb_prev = None
            for hb in range(NBAND):
                xb = conv1_band(hb)
                if xb_prev is not None:
                    tail_band(hb - 1, xb_prev)
                xb_prev = xb
            tail_band(NBAND - 1, xb_prev)
    nc.compile()
    return nc


_NC_CACHE = {}


def _get_nc(with_b2: bool, with_b1: bool = True):
    key = (with_b2, with_b1)
    if key not in _NC_CACHE:
        _NC_CACHE[key] = _build(with_b2, with_b1=with_b1)
    return _NC_CACHE[key]


def _prep_core(feat_b, flow_b):
    """Per-core input arrays from one batch element."""
    # feat: [256, 96, 96] -> padded grid [128p, 2kc, 100*98] with one extra
    # leading zero row (grid row r lives at flat (r+1)*98).
    featp = np.zeros((2, 128, 100 * GW), np.float32)
    grid = np.zeros((C, GW, GW), np.float32)
    grid[:, 1 : H + 1, 1 : W + 1] = feat_b
    featp[:, :, GW : GW + GW * GW] = grid.reshape(2, 128, GW * GW)
    featp = _mm_np(featp.transpose(1, 0, 2))

    # flow unfold (zero-padded 3x3 shifts), pre-scaled by UP=8:
    # fdat[i, j, c*9+k] = 8 * flow_pad[c, h+kh, w+kw] for pixel j*128+i
    fpad = np.zeros((2, H + 2, W + 2), np.float32)
    fpad[:, 1:-1, 1:-1] = flow_b
    shifts = np.stack(
        [fpad[:, i : i + H, j : j + W] for i in range(3) for j in range(3)],
        axis=1,
    )  # [2, 9, 96, 96]
    fdat = (8.0 * shifts).transpose(2, 3, 0, 1).reshape(PIX, 18)
    fdat = np.ascontiguousarray(fdat.reshape(NCHUNK, 128, 18).transpose(1, 0, 2))
    return featp, fdat


def _weight_args(inputs):
    """Re-laid-out weight arrays shared across cores."""
    w1 = np.asarray(inputs["w1"], np.float32)
    b1 = np.asarray(inputs["b1"], np.float32)
    w2 = np.asarray(inputs["w2"], np.float32)
    b2 = np.asarray(inputs["b2"], np.float32)
    # w1s[ci_in, kc, tap, mc, co_in] = w1[mc*128+co, kc*128+ci, kh, kw]
    t = w1.transpose(1, 2, 3, 0).reshape(2, 128, 3, 3, 2, 128)
    w1s = _mm_np(t.transpose(1, 0, 2, 3, 4, 5)).reshape(
        128, 2, 9, 2, 128
    )
    # w2 columns reordered to ch' = pq*9 + k (from ch = k*64 + pq)
    chp = np.arange(576)
    perm = (chp % 9) * 64 + chp // 9
    w2m = w2[:, :, 0, 0][perm]  # [576, 256]
    w2s = _mm_np(w2m.T.reshape(2, 128, 576).transpose(1, 0, 2))
    b1s = np.ascontiguousarray(b1.reshape(2, 128).T)
    args = {"w1s": w1s, "w2s": w2s, "b1s": b1s}
    if np.any(b2):
        args["b2s"] = _mm_np(b2[perm].reshape(1, 576))
    return args


def kernel(flow_lr, feat, w1, b1, w2, b2):
    flow_lr = np.asarray(flow_lr, np.float32)
    feat = np.asarray(feat, np.float32)
    inputs = {"w1": w1, "b1": b1, "w2": w2, "b2": b2}
    w_args = _weight_args(inputs)
    with_b2 = "b2s" in w_args
    with_b1 = bool(np.any(np.asarray(b1)))

    nc = _get_nc(with_b2, with_b1)
    in_maps = []
    for b in range(B):
        featp, fdat = _prep_core(feat[b], flow_lr[b])
        m = dict(w_args)
        m.update({"featp": featp, "fdat": fdat})
        in_maps.append(m)

    res = run_bass_kernel_spmd(nc, in_maps, list(range(B)))
    return np.stack([res.results[b]["out"] for b in range(B)]).astype(np.float32)
